# revision 21
# baseline (speedup 1.0000x reference)
"""Bahdanau additive attention on 8 Trainium2 NeuronCores.

reference:
    eh = enc @ W_h.T            [B,S,H]
    qs = q   @ W_s.T            [B,T,H]
    score[b,t,s] = sum_h v[h] * tanh(eh[b,s,h] + qs[b,t,h])
    score = where(mask, score, -inf); attn = softmax_s(score)
    ctx[b,t,:] = sum_s attn[b,t,s] * enc[b,s,:]

Sharding: data-parallel over batch B=8, one batch per NeuronCore.

Algorithm: the tanh over the [T,S,H] broadcast-sum is the dominant cost
(33.5M scalar-engine lookups/core ~ 218us). Instead we expand

    tanh(x) ~= alpha*x + sum_m c_m sin(m*w1*x)      (weighted LSQ fit)

and use sin(m*w1*(a+b)) = sin(m*w1*a)cos(m*w1*b) + cos(m*w1*a)sin(m*w1*b),
which turns the score into a plain matmul over an expanded contraction
dim (2M*H) that the PE array handles at full fp16 rate:

    score[t,s] = sum_m sum_h [v_h*sin_m(eh)][h,s] * [c_m*cos_m(qs)][h,t]
                            + [v_h*cos_m(eh)][h,s] * [c_m*sin_m(qs)][h,t]
                 + alpha * (v . eh)[s]   (+ t-only terms: softmax-invariant)

Per-core device program:
  - PE: ehT[d,s] = whT.T @ encT, qsT[d,t] = wsT.T @ qT  (fp16, PSUM fp32)
  - DVE: clamp projections to [-4.4, 4.4] (keeps all sin args in the
    fitted/periodic domain), cast fp16
  - ACT Sin: seed harmonics m=1,2 (args <= pi, where the HW spline is
    exact); DVE Chebyshev recurrence s_{m+1} = 2cos(w1 a) s_m - s_{m-1}
    for m=3..8 in fp16 2x mode (F-side seeds pre-scaled by v; linearity
    propagates the scale, G-side scaled by c_m per feature)
  - PE: 16 pair-matmuls accumulate score[t,s] in PSUM (+ mask penalty and
    the alpha*(v.eh) rank-1 term seeded via 1-row matmuls)
  - softmax over s on the free axis (DVE reduce_max / ACT exp+accum /
    DVE reciprocal), normalization folded into the context epilogue
  - PE transposes attn (fp16), context matmul against enc natural layout
"""

import sys

try:
    import concourse.bass as bass  # noqa: F401
except ImportError:  # pragma: no cover
    sys.path.insert(0, "/opt/trn_rl_repo")

import numpy as np

import concourse.bass as bass
import concourse.bacc as bacc
import concourse.mybir as mybir
from concourse import tile

FP32 = mybir.dt.float32
FP16 = mybir.dt.float16

N_CORES = 8
H = 512
T_FULL = 256
S_FULL = 256

# ---- tanh ~ alpha*x + sum_m cm sin(m*w1*x) fit (weighted LSQ) ----
M_HARM = 6
CLAMP = 4.4
W1 = np.pi / 9.0


def _fit_tanh_sine():
    xs = np.linspace(-2 * CLAMP, 2 * CLAMP, 8001)
    wgt = np.exp(-xs ** 2 / 4.0) + 1e-3
    A = np.concatenate(
        [xs[:, None], np.sin(np.outer(xs, np.arange(1, M_HARM + 1) * W1))], axis=1)
    Aw = A * wgt[:, None]
    coef = np.linalg.lstsq(Aw.T @ A, Aw.T @ np.tanh(xs), rcond=None)[0]
    return float(coef[0]), [float(c) for c in coef[1:]]


ALPHA, CM = _fit_tanh_sine()


def build_program(T=T_FULL, S=S_FULL, n_cores=N_CORES, nrep=1, debug=False, ablate=()):
    """Build the per-core Bass program. T/S parametrized for cheap sim runs."""
    assert H % 128 == 0 and T % 128 == 0 and S % 128 == 0
    DC = H // 128          # h chunks
    TB = T // 128          # t blocks (partition blocks of the score)
    SB = S // 128          # s blocks
    M = M_HARM

    nc = bacc.Bacc("TRN2", target_bir_lowering=False, debug=debug,
                   num_devices=n_cores)

    encT_d = nc.dram_tensor("encT", [H, S], FP16, kind="ExternalInput")
    enc_d = nc.dram_tensor("enc", [S, H], FP16, kind="ExternalInput")
    qT_d = nc.dram_tensor("qT", [H, T], FP16, kind="ExternalInput")
    whT_d = nc.dram_tensor("whT", [H, H], FP16, kind="ExternalInput")
    wsT_d = nc.dram_tensor("wsT", [H, H], FP16, kind="ExternalInput")
    vq_d = nc.dram_tensor("vq", [128, DC], FP32, kind="ExternalInput")
    av_d = nc.dram_tensor("av", [128, DC], FP16, kind="ExternalInput")
    pen_d = nc.dram_tensor("pen", [1, S], FP16, kind="ExternalInput")
    ones_d = nc.dram_tensor("ones", [1, 128], FP16, kind="ExternalInput")
    ident_d = nc.dram_tensor("ident", [128, 128], FP16, kind="ExternalInput")
    phase_d = nc.dram_tensor("phase", [128, 1], FP32, kind="ExternalInput")
    ctx_d = nc.dram_tensor("ctx", [T, H], FP32, kind="ExternalOutput")

    SIN = mybir.ActivationFunctionType.Sin
    EXP = mybir.ActivationFunctionType.Exp
    IDN = mybir.ActivationFunctionType.Identity

    with tile.TileContext(nc) as tc:
        with (
            tc.tile_pool(name="const", bufs=1) as const_pool,
            tc.tile_pool(name="work", bufs=1) as work_pool,
            tc.tile_pool(name="feat", bufs=1) as feat_pool,
            tc.tile_pool(name="tmp", bufs=4) as tmp_pool,
            tc.tile_pool(name="post", bufs=1) as post_pool,
            tc.tile_pool(name="ppsum", bufs=2, space=bass.MemorySpace.PSUM) as ppsum,
            tc.tile_pool(name="upsum", bufs=1, space=bass.MemorySpace.PSUM) as upsum,
            tc.tile_pool(name="spsum", bufs=1, space=bass.MemorySpace.PSUM) as spsum,
            tc.tile_pool(name="apsum", bufs=1, space=bass.MemorySpace.PSUM) as apsum,
            tc.tile_pool(name="cpsum", bufs=1, space=bass.MemorySpace.PSUM) as cpsum,
        ):
            # ---- load constants / inputs (few big DMAs; critical first) ----
            whT_cat = const_pool.tile([128, 4 * H], FP16, name="whT_cat")
            wsT_cat = const_pool.tile([128, 4 * H], FP16, name="wsT_cat")
            encT_cat = const_pool.tile([128, DC * S], FP16, name="encT_cat")
            qT_cat = const_pool.tile([128, DC * T], FP16, name="qT_cat")
            enc_cat = const_pool.tile([128, SB * H], FP16, name="enc_cat")
            vq_sb = const_pool.tile([128, DC], FP32, tag="vq")
            av_sb = const_pool.tile([128, DC], FP16, tag="av")
            pen_sb = const_pool.tile([1, S], FP16, tag="pen")
            ones_sb = const_pool.tile([1, 128], FP16, tag="ones")
            ident_sb = const_pool.tile([128, 128], FP16, tag="ident")
            phase_sb = const_pool.tile([128, 1], FP32, tag="phase")

            def cat_load(dst, src_d, blocks):
                nc.sync.dma_start(
                    dst[:].rearrange("p (a j) -> p a j", a=blocks),
                    src_d.rearrange("(a p) j -> p a j", p=128),
                )

            cat_load(whT_cat, whT_d, 4)
            cat_load(encT_cat, encT_d, DC)
            cat_load(wsT_cat, wsT_d, 4)
            cat_load(qT_cat, qT_d, DC)
            nc.sync.dma_start(vq_sb[:], vq_d[:])
            nc.sync.dma_start(av_sb[:], av_d[:])
            nc.sync.dma_start(pen_sb[:], pen_d[:])
            nc.sync.dma_start(ones_sb[:], ones_d[:])
            nc.sync.dma_start(phase_sb[:], phase_d[:])
            cat_load(enc_cat, enc_d, SB)
            nc.sync.dma_start(ident_sb[:], ident_d[:])

            whT_sb = [whT_cat[:, H * i:H * (i + 1)] for i in range(4)]
            wsT_sb = [wsT_cat[:, H * i:H * (i + 1)] for i in range(4)]
            encT_sb = [encT_cat[:, S * i:S * (i + 1)] for i in range(DC)]
            qT_sb = [qT_cat[:, T * i:T * (i + 1)] for i in range(DC)]
            enc_sb = [enc_cat[:, H * i:H * (i + 1)] for i in range(SB)]

            import contextlib

            # Depth-2 software pipeline: features are produced one slot
            # ahead of the score matmuls that read them (double-buffered via
            # tag suffix), so PE never waits on the DVE chains. score_ps is
            # consumed (softmax) at the top of the next slot before refill.
            score_ps = [spsum.tile([128, S], FP32, name=f"score{tb}",
                                   tag=f"score{tb}") for tb in range(TB)]

            def alloc_feat_set(sfx):
                Fd, Gd = {}, {}
                for m in range(1, M + 1):
                    for t_ in ("s", "c"):
                        Fd[(t_, m)] = feat_pool.tile(
                            [128, DC * S], FP16, tag=f"F{t_}{m}{sfx}",
                            name=f"F{t_}{m}{sfx}")
                        Gd[(t_, m)] = feat_pool.tile(
                            [128, DC * T], FP16, tag=f"G{t_}{m}{sfx}",
                            name=f"G{t_}{m}{sfx}")
                upen = post_pool.tile([1, S], FP16, tag=f"upen{sfx}")
                return Fd, Gd, upen

            def emit_features(fs):
                F, G, upen = fs
                # ---- projections -> clamped fp16 ehT/qsT [128, DC*{S,T}] ----
                ehT = work_pool.tile([128, DC * S], FP16, tag="ehT")
                qsT = work_pool.tile([128, DC * T], FP16, tag="qsT")

                def project(wT_sb, xT_sb, dst, N):
                    for dc in range(DC):
                        ps = ppsum.tile([128, N], FP32, tag="proj_ps")
                        for hc in range(4):
                            nc.tensor.matmul(
                                ps[:],
                                wT_sb[hc][:, 128 * dc:128 * (dc + 1)],
                                xT_sb[hc],
                                start=(hc == 0), stop=(hc == 3),
                            )
                        nc.vector.tensor_scalar(
                            dst[:, N * dc:N * (dc + 1)], ps[:],
                            CLAMP, -CLAMP,
                            mybir.AluOpType.min, mybir.AluOpType.max,
                        )

                project(whT_sb, encT_sb, ehT, S)
                project(wsT_sb, qT_sb, qsT, T)

                # ---- u[s] = alpha * (v . eh)[s]; upen = u + pen ----
                u_ps = upsum.tile([1, S], FP32, tag="u_ps")
                for dc in range(DC):
                    nc.tensor.matmul(
                        u_ps[:], av_sb[:, dc:dc + 1], ehT[:, S * dc:S * (dc + 1)],
                        start=(dc == 0), stop=(dc == DC - 1),
                    )
                nc.vector.tensor_add(upen[:], u_ps[:], pen_sb[:])

                # ---- ACT seed harmonics: m=1,2 sin/cos plus direct m=3 sin
                # (3*w1*4.4 = 4.6 rad is still within the usable HW Sin range;
                # the rare tail error is noise-level after the v-weighted sum)
                ehs = {}  # raw eh-side trig
                qss = {}  # raw qs-side trig
                for m in (1, 2):
                    for (d, nm) in ((ehs, "e"), (qss, "q")):
                        src = ehT if nm == "e" else qsT
                        s_t = feat_pool.tile([128, DC * S], FP16, tag=f"{nm}s{m}r")
                        c_t = feat_pool.tile([128, DC * S], FP16, tag=f"{nm}c{m}r")
                        nc.scalar.activation(s_t[:], src[:], SIN, scale=m * W1)
                        nc.scalar.activation(c_t[:], src[:], SIN, scale=m * W1,
                                             bias=phase_sb[:])
                        d[("s", m)] = s_t
                        d[("c", m)] = c_t
                for (d, nm) in ((ehs, "e"), (qss, "q")):
                    src = ehT if nm == "e" else qsT
                    s3_t = feat_pool.tile([128, DC * S], FP16, tag=f"{nm}s3r")
                    nc.scalar.activation(s3_t[:], src[:], SIN, scale=3 * W1)
                    d[("s", 3)] = s3_t

                # chain multipliers 2*cos(w1*a), unscaled
                Cch_e = feat_pool.tile([128, DC * S], FP16, tag="Cch_e")
                Cch_q = feat_pool.tile([128, DC * S], FP16, tag="Cch_q")
                nc.vector.tensor_scalar_mul(Cch_e[:], ehs[("c", 1)][:], 2.0)
                nc.vector.tensor_scalar_mul(Cch_q[:], qss[("c", 1)][:], 2.0)

                # ---- F chain seeds: v-scaled copies of eh seeds ----
                for (t_, m) in (("s", 1), ("c", 1), ("s", 2), ("c", 2), ("s", 3)):
                    ft = F[(t_, m)]
                    for dc in range(DC):
                        nc.vector.tensor_scalar_mul(
                            ft[:, S * dc:S * (dc + 1)],
                            ehs[(t_, m)][:, S * dc:S * (dc + 1)],
                            vq_sb[:, dc:dc + 1],
                        )

                # ---- G scaled seed features: cm * qs seeds (GPSIMD) ----
                for (t_, m) in (("s", 1), ("c", 1), ("s", 2), ("c", 2), ("s", 3)):
                    nc.gpsimd.tensor_scalar_mul(
                        G[(t_, m)][:], qss[(t_, m)][:], CM[m - 1])

                # ---- Chebyshev recurrence + G scaling + matmuls ----
                # s-chains start from the direct s3 seed; c-chains from c1,c2
                qchain = {("s", 2): qss[("s", 2)], ("c", 1): qss[("c", 1)],
                          ("s", 3): qss[("s", 3)], ("c", 2): qss[("c", 2)]}
                for m in range(3, M + 1):
                    for t_ in ("s", "c"):
                        if not (m == 3 and t_ == "s"):
                            # F side: chain is v-scaled (linear recurrence)
                            ft = F[(t_, m)]
                            tmp = tmp_pool.tile([128, DC * S], FP16, tag="rectmp")
                            nc.vector.tensor_mul(tmp[:], Cch_e[:], F[(t_, m - 1)][:])
                            nc.vector.tensor_sub(ft[:], tmp[:], F[(t_, m - 2)][:])
                            # G side: raw chain + cm-scaled feature copy
                            qt = feat_pool.tile([128, DC * T], FP16, tag=f"q{t_}{m}r")
                            tmp2 = tmp_pool.tile([128, DC * T], FP16, tag="rectmp")
                            nc.vector.tensor_mul(tmp2[:], Cch_q[:], qchain[(t_, m - 1)][:])
                            nc.vector.tensor_sub(qt[:], tmp2[:], qchain[(t_, m - 2)][:])
                            qchain[(t_, m)] = qt
                            nc.gpsimd.tensor_scalar_mul(
                                G[(t_, m)][:], qt[:], CM[m - 1])
            def emit_scores(fs):
                F, G, upen = fs
                # ---- score PSUM: seed with pen+u, accumulate all pairs ----
                for tb in range(TB):
                    nc.tensor.matmul(
                        score_ps[tb][:], ones_sb[:], upen[:],
                        start=True, stop=False, skip_group_check=True,
                    )
                for m in range(1, M + 1):
                    for pi, (gt, ft) in enumerate(
                            ((G[("c", m)], F[("s", m)]),
                             (G[("s", m)], F[("c", m)]))):
                        for hc in range(DC):
                            for tb in range(TB):
                                nc.tensor.matmul(
                                    score_ps[tb][:],
                                    gt[:, T * hc + 128 * tb:T * hc + 128 * (tb + 1)],
                                    ft[:, S * hc:S * (hc + 1)],
                                    start=False,
                                    stop=(m == M and pi == 1 and hc == DC - 1),
                                    skip_group_check=True,
                                )

            def emit_consume():
                # ---- softmax over s (free axis) ----
                attn_sb = [post_pool.tile([128, S], FP16, name=f"attn{tb}",
                                          tag=f"attn{tb}") for tb in range(TB)]
                rden = [post_pool.tile([128, 1], FP32, name=f"rden{tb}",
                                       tag=f"rden{tb}") for tb in range(TB)]
                for tb in range(TB):
                    nmax = post_pool.tile([128, 1], FP32, tag=f"nmax{tb}")
                    nc.vector.reduce_max(
                        nmax[:], score_ps[tb][:],
                        axis=mybir.AxisListType.X, negate=True)
                    den = post_pool.tile([128, 1], FP32, tag=f"den{tb}")
                    nc.scalar.activation(
                        attn_sb[tb][:], score_ps[tb][:], EXP,
                        bias=nmax[:], scale=1.0, accum_out=den[:])
                    nc.vector.reciprocal(rden[tb][:], den[:])

                # ---- transpose attn -> attnT (fp16) ----
                attnT_sb = [post_pool.tile([128, T], FP16, name=f"attnT{sb}",
                                           tag=f"attnT{sb}") for sb in range(SB)]
                for sb in range(SB):
                    at_ps = apsum.tile([128, T], FP16, tag="at_ps")
                    for tb in range(TB):
                        nc.tensor.transpose(
                            at_ps[:, 128 * tb:128 * (tb + 1)],
                            attn_sb[tb][:, 128 * sb:128 * (sb + 1)],
                            ident_sb[:],
                        )
                    nc.scalar.activation(attnT_sb[sb][:], at_ps[:], IDN)

                # ---- context: ctx[t, :] = sum_s attn[t,s] enc[s,:] ----
                for tb in range(TB):
                    ctx_ps = cpsum.tile([128, H], FP32, tag="ctx_ps")
                    for sb in range(SB):
                        nc.tensor.matmul(
                            ctx_ps[:],
                            attnT_sb[sb][:, 128 * tb:128 * (tb + 1)],
                            enc_sb[sb],
                            start=(sb == 0), stop=(sb == SB - 1),
                        )
                    ctx_sb = post_pool.tile([128, H], FP32, tag=f"ctx{tb}")
                    nc.scalar.activation(
                        ctx_sb[:], ctx_ps[:], IDN, scale=rden[tb][:])
                    nc.sync.dma_start(ctx_d[128 * tb:128 * (tb + 1), :], ctx_sb[:])

            # pipeline fill: features(0) -> A, scores(0), features(1) -> B
            fA = alloc_feat_set("A")
            fB = alloc_feat_set("B")
            emit_features(fA)
            emit_scores(fA)
            emit_features(fB)
            if nrep == 1:
                emit_consume()
            else:
                # partial unroll: the For_i back-edge serializes engines
                # (~8us), so amortize it over UNROLL iterations per trip
                UNROLL = 8
                assert nrep % UNROLL == 0, "nrep must be a multiple of UNROLL"
                with tc.For_i(0, nrep // UNROLL, 1):
                    for _ in range(UNROLL // 2):
                        emit_consume()
                        emit_scores(fB)
                        emit_features(fA)
                        emit_consume()
                        emit_scores(fA)
                        emit_features(fB)

    nc.compile()
    return nc


def make_in_maps(encoder_outputs, query, mask, W_h, W_s, v, T=T_FULL, S=S_FULL):
    B = encoder_outputs.shape[0]
    DC = H // 128
    whT = np.ascontiguousarray(W_h.astype(np.float32).T.astype(np.float16))
    wsT = np.ascontiguousarray(W_s.astype(np.float32).T.astype(np.float16))
    v32 = v.astype(np.float32)
    vq = np.ascontiguousarray(v32.reshape(DC, 128).T)                 # [128, DC] fp32
    av = np.ascontiguousarray((ALPHA * v32).reshape(DC, 128).T.astype(np.float16))
    ones = np.ones((1, 128), np.float16)
    ident = np.eye(128, dtype=np.float16)
    phase = np.full((128, 1), np.pi / 2, np.float32)
    in_maps = []
    for b in range(B):
        enc_b = np.ascontiguousarray(encoder_outputs[b].astype(np.float32))
        q_b = query[b].astype(np.float32)
        pen = np.where(mask[b], 0.0, -3.0e4).astype(np.float16).reshape(1, S)
        in_maps.append({
            "encT": np.ascontiguousarray(enc_b.T.astype(np.float16)),
            "enc": enc_b.astype(np.float16),
            "qT": np.ascontiguousarray(q_b.T.astype(np.float16)),
            "whT": whT,
            "wsT": wsT,
            "vq": vq,
            "av": av,
            "pen": pen,
            "ones": ones,
            "ident": ident,
            "phase": phase,
        })
    return in_maps


_PROGRAM_CACHE = {}


def kernel(encoder_outputs, query, mask, W_h, W_s, v):
    from concourse.bass_utils import run_bass_kernel_spmd

    B = encoder_outputs.shape[0]
    assert B == N_CORES
    key = (T_FULL, S_FULL, N_CORES)
    if key not in _PROGRAM_CACHE:
        _PROGRAM_CACHE[key] = build_program()
    nc = _PROGRAM_CACHE[key]
    in_maps = make_in_maps(encoder_outputs, query, mask, W_h, W_s, v)
    res = run_bass_kernel_spmd(nc, in_maps, list(range(N_CORES)))
    out = np.stack([res.results[b]["ctx"] for b in range(B)], axis=0)
    return out.astype(np.float32)
